# revision 42
# baseline (speedup 1.0000x reference)
"""Trainium2 Bass kernel for nn_CasualGraph (segment_reduce).

Computes, on 8 NeuronCores:
    last = x0
    for l in range(num_layers):
        t      = A @ last
        source = A.T @ t
        last   = LN(source + x0)
    Hb    = (H > 0)
    means = (Hb.T @ source) / Hb.sum(0)[:, None]
    out   = means.max(axis=0)            # [D]

Sharding (8 cores, data-parallel over node rows):
  core i owns rows_i = rows [i*N/8, (i+1)*N/8).
  - pass1: t[rows_i] = A[rows_i, :] @ last    (lhsT slabs produced by DMA-transpose
           of a bf16, column-chunked copy of the row shard staged in DRAM)
  - AllGather(t)  [bf16, N/8 x D per rank]
  - pass2: source[rows_i] = A[:, rows_i].T @ t  (bf16 column shard resident in SBUF)
  - LN + residual locally, AllGather(last)
  Final layer gathers `source` instead, then each core computes the masked-mean
  over its own E/8 hyperedge shard ([source | 1].T @ H on PE) and the cross-core
  max is an AllReduce(max).

All matmuls run in bf16 (1 cycle/row on PE vs 4 for fp32) with fp32 PSUM
accumulation; numpy-simulated max relative error vs the fp32 reference is ~2.6e-3.
"""

import os
import sys

for _p in ("/opt/trn_rl_repo", os.path.expanduser("~/.axon_site/_ro/trn_rl_repo")):
    if os.path.isdir(_p) and _p not in sys.path:
        sys.path.insert(0, _p)

import ml_dtypes
import numpy as np

from concourse import bacc, bass, bass_utils, mybir, tile

F32 = mybir.dt.float32
BF16 = mybir.dt.bfloat16
P = 128  # SBUF/PSUM partitions


def build_program(N, D, E, n_layers, n_cores):
    """Build the SPMD Bass/Tile program (same program on every core)."""
    Nl = N // n_cores      # local rows per core
    MB = Nl // P           # output row-blocks per core
    KB = N // P            # contraction blocks over full N
    CH = 8                 # k-blocks per rhs streaming chunk
    NCH = KB // CH
    El = E // n_cores      # hyperedge shard
    DB = D // P            # D row-blocks (for the transposed segment-sum)
    assert Nl % P == 0 and N % P == 0 and KB % CH == 0 and D % P == 0
    eps = 1e-5

    QW = 256               # a_row columns per setup chunk
    NQ = N // QW
    KPQ = QW // P          # k-blocks per setup chunk
    NH = MB // 2           # PSUM accumulation tiles (2 row-blocks per bank)
    assert MB % 2 == 0

    nc = bacc.Bacc(
        "TRN2", target_bir_lowering=False, debug=False, num_devices=n_cores
    )
    ident_np = np.eye(P, dtype=np.float32)
    ident_dram = nc.inline_tensor(
        ident_np.astype(ml_dtypes.bfloat16), name="ident"
    )
    ident_f32_dram = nc.inline_tensor(ident_np, name="ident_f32")

    a_col = nc.dram_tensor("a_col", [N, Nl], F32, kind="ExternalInput").ap()
    a_row = nc.dram_tensor("a_row", [Nl, N], F32, kind="ExternalInput").ap()
    x0_full = nc.dram_tensor("x0_full", [N, D], F32, kind="ExternalInput").ap()
    x0_loc = nc.dram_tensor("x0_loc", [Nl, D], F32, kind="ExternalInput").ap()
    hcol = nc.dram_tensor("hcol", [N, El], F32, kind="ExternalInput").ap()
    gamma = nc.dram_tensor("gamma", [D], F32, kind="ExternalInput").ap()
    beta = nc.dram_tensor("beta", [D], F32, kind="ExternalInput").ap()
    out = nc.dram_tensor("out", [D], F32, kind="ExternalOutput").ap()

    rg = [list(range(n_cores))]
    bypass = mybir.AluOpType.bypass
    add = mybir.AluOpType.add
    mult = mybir.AluOpType.mult
    amax = mybir.AluOpType.max
    AX = mybir.AxisListType.X
    ACT = mybir.ActivationFunctionType

    with tile.TileContext(nc) as tc:
        with (
            tc.tile_pool(name="dram", bufs=1, space="DRAM") as dpool,
            tc.tile_pool(name="const", bufs=1) as cpool,
            tc.tile_pool(name="acolp", bufs=1) as acol_pool,
            tc.tile_pool(name="stream", bufs=1) as spool,
            tc.tile_pool(name="psum", bufs=1, space="PSUM") as ppool,
        ):
            # ---------------- DRAM staging buffers ----------------
            # Pre-transposed bf16 row shard: at_dram[kb] = A[rows_i, kb*P:(kb+1)*P].T
            # stored as contiguous [P, Nl] slabs (pass1 lhsT, plain streaming reads).
            at_dram = dpool.tile([KB // 2, P, 2 * Nl], BF16, name="at_dram")
            x0_bf = dpool.tile([N, D], BF16, name="x0_bf")
            # Shared collective outputs must each have a single writer:
            # one AllGather output buffer per collective.
            t_ag_in = dpool.tile([Nl, D], BF16, name="t_ag_in")
            t_ag_out = [
                dpool.tile(
                    [N, D], BF16, name=f"t_ag_out_{l}", addr_space="Shared"
                )
                for l in range(n_layers)
            ]
            last_ag_in = dpool.tile([Nl, D], BF16, name="last_ag_in")
            last_ag_out = [
                dpool.tile(
                    [N, D], BF16, name=f"last_ag_out_{l}", addr_space="Shared"
                )
                for l in range(n_layers - 1)
            ]
            src_ag_in = dpool.tile([Nl, D], BF16, name="src_ag_in")
            src_full = dpool.tile(
                [N, D], BF16, name="src_full", addr_space="Shared"
            )
            h_bf = dpool.tile([N, El], BF16, name="h_bf")
            armax_in = dpool.tile([1, D], F32, name="armax_in")
            armax_out = dpool.tile(
                [1, D], F32, name="armax_out", addr_space="Shared"
            )

            # ---------------- constants ----------------
            ident = cpool.tile([P, P], BF16, name="ident")
            nc.sync.dma_start(ident[:], ident_dram.ap())
            ident_f32 = cpool.tile([P, P], F32, name="ident_f32")
            nc.sync.dma_start(ident_f32[:], ident_f32_dram.ap())
            x0_sb = cpool.tile([P, MB, D], F32, name="x0_sb")
            nc.scalar.dma_start(
                x0_sb[:], x0_loc.rearrange("(mb p) d -> p mb d", p=P)
            )
            gb_row = cpool.tile([1, 2 * D], F32, name="gb_row")
            nc.scalar.dma_start(gb_row[:, 0:D], gamma[None, :])
            nc.scalar.dma_start(gb_row[:, D : 2 * D], beta[None, :])
            gb_sb = cpool.tile([P, 2 * D], F32, name="gb_sb")
            nc.gpsimd.partition_broadcast(gb_sb[:], gb_row[:])
            gamma_sb = gb_sb[:, 0:D]
            beta_sb = gb_sb[:, D : 2 * D]
            ones_sb = cpool.tile([P, 1], BF16, name="ones_sb")
            nc.vector.memset(ones_sb[:], 1.0)
            eps_sb = cpool.tile([P, 1], F32, name="eps_sb")
            nc.vector.memset(eps_sb[:], eps)

            # ---------------- setup casts (SWDGE FIFO order matters) ----
            # x0 (bf16) is layer-0 "last": gates the first rhs chunks
            nc.gpsimd.dma_start(x0_bf[:], x0_full[:])

            def load_rhs(src_r, c, tag):
                rhs = spool.tile(
                    [P, CH, D], BF16, name="rhs", tag="rhs", bufs=3
                )
                nc.scalar.dma_start(rhs[:], src_r[:, c, :, :])
                return rhs

            acol_sb = acol_pool.tile([P, KB, Nl], BF16, name="acol_sb")
            a_col_r = a_col.rearrange("(kb p) m -> p kb m", p=P)

            # ---------------- propagation layers ----------------
            # Both passes run "flipped": the stationary operand is the small
            # [128, D] rhs-chunk half (2 LDWEIGHTS per k-block) and the moving
            # operand is the wide A slab (N=512), producing transposed outputs
            # t.T / source.T in two [P, Nl] psum tiles (one accumulation group
            # per bank). Outputs are PE-transposed back to natural layout.
            # Layer 0's pass1 is fused with setup: a_row is cast-loaded in
            # column chunks and PE-transposed into lhsT slabs that feed both
            # at_dram (for layers 1+) and layer-0's matmuls directly.
            MH = max(Nl // 512, 1)
            MW = Nl // MH

            def pass_mms(tps, slab, rhs, j, kb):
                for dh in range(DB):
                    for mh in range(MH):
                        nc.tensor.matmul(
                            tps[dh][:, mh * MW : (mh + 1) * MW],
                            rhs[:, j, dh * P : (dh + 1) * P],
                            slab[:, mh * MW : (mh + 1) * MW],
                            start=(kb == 0),
                            stop=(kb == KB - 1),
                        )

            for l in range(n_layers):
                is_last = l == n_layers - 1

                # ---- pass1: t[rows_i] = A[rows_i, :] @ last  (as t.T) ----
                tps1 = [
                    ppool.tile([P, Nl], F32, name=f"tps1_{dh}", tag=f"ps_t{dh}")
                    for dh in range(DB)
                ]
                last_in = x0_bf if l == 0 else last_ag_out[l - 1]
                last_r = last_in.rearrange("(c j p) d -> p c j d", p=P, j=CH)
                rhs1 = None
                if l == 0:
                    # Stage a_row as bf16 column blocks (SWDGE casts,
                    # interleaved with the acol resident casts), then
                    # xbar-DMA-transpose each [Nl, P] column into an SBUF
                    # slab that feeds at_dram staging AND layer-0's matmuls.
                    NCB = 8
                    CBW = N // NCB
                    KPB = CBW // P
                    arow_bfb = [
                        dpool.tile([Nl, CBW], BF16, name=f"arow_bf{qb}")
                        for qb in range(NCB)
                    ]
                    for qb in range(NCB):
                        nc.gpsimd.dma_start(
                            arow_bfb[qb][:],
                            a_row[:, qb * CBW : (qb + 1) * CBW],
                        )
                        sl = slice(qb * (KB // NCB), (qb + 1) * (KB // NCB))
                        nc.gpsimd.dma_start(
                            acol_sb[:, sl, :], a_col_r[:, sl, :]
                        )
                    # H shard -> bf16 staging; runs on the idle SWDGE queue
                    # during the layers so the hyperedge stage reads bf16
                    nc.gpsimd.dma_start(h_bf[:], hcol[:])
                    for kb in range(KB):
                        if kb % CH == 0:
                            rhs1 = load_rhs(last_r, kb // CH, "rhs1")
                        slab = spool.tile(
                            [P, Nl], BF16, name="slab", tag="slab", bufs=3
                        )
                        qb, kbl = divmod(kb, KPB)
                        nc.sync.dma_start(
                            slab[:],
                            arow_bfb[qb][:, kbl * P : (kbl + 1) * P],
                            transpose=True,
                        )
                        nc.scalar.dma_start(
                            at_dram[kb // 2][
                                :, (kb % 2) * Nl : (kb % 2 + 1) * Nl
                            ],
                            slab[:],
                        )
                        pass_mms(tps1, slab, rhs1, kb % CH, kb)
                else:
                    for kb2 in range(KB // 2):
                        kb0 = kb2 * 2
                        if kb0 % CH == 0:
                            rhs1 = load_rhs(last_r, kb0 // CH, "rhs1")
                        slab2 = spool.tile(
                            [P, 2 * Nl], BF16, name="slab2", tag="slab2",
                            bufs=3,
                        )
                        nc.sync.dma_start(slab2[:], at_dram[kb2])
                        for q2 in range(2):
                            kb = kb0 + q2
                            pass_mms(
                                tps1,
                                slab2[:, q2 * Nl : (q2 + 1) * Nl],
                                rhs1,
                                kb % CH,
                                kb,
                            )

                # transpose t.T back to natural bf16 tiles and AllGather
                t_loc = spool.tile([P, MB, D], BF16, name="t_loc", tag="t_loc")
                tT_sb = [
                    spool.tile([P, Nl], BF16, name="tTs", tag="tTs", bufs=2)
                    for _ in range(DB)
                ]
                for dh in range(DB):
                    nc.vector.tensor_copy(tT_sb[dh][:], tps1[dh][:])
                for mb in range(MB):
                    for dh in range(DB):
                        tr = ppool.tile(
                            [P, P], BF16, name="trb", tag="ps_tr", bufs=4
                        )
                        nc.tensor.transpose(
                            tr[:],
                            tT_sb[dh][:, mb * P : (mb + 1) * P],
                            ident[:],
                        )
                        nc.vector.tensor_copy(
                            t_loc[:, mb, dh * P : (dh + 1) * P], tr[:]
                        )
                nc.scalar.dma_start(
                    t_ag_in.rearrange("(mb p) d -> p mb d", p=P), t_loc[:]
                )
                nc.gpsimd.collective_compute(
                    "AllGather",
                    bypass,
                    replica_groups=rg,
                    ins=[t_ag_in[:].opt()],
                    outs=[t_ag_out[l][:].opt()],
                )

                # ---- pass2: source[rows_i] = A[:, rows_i].T @ t (as src.T) ----
                tps2 = [
                    ppool.tile([P, Nl], F32, name=f"tps2_{dh}", tag=f"ps_t{dh}")
                    for dh in range(DB)
                ]
                t_r = t_ag_out[l].rearrange("(c j p) d -> p c j d", p=P, j=CH)
                rhs2 = None
                for kb in range(KB):
                    if kb % CH == 0:
                        rhs2 = load_rhs(t_r, kb // CH, "rhs2")
                    pass_mms(tps2, acol_sb[:, kb, :], rhs2, kb % CH, kb)

                # transpose src.T back (bf16), fusing the +x0 residual
                sT_sb = [
                    spool.tile([P, Nl], BF16, name="sTs", tag="sTs", bufs=2)
                    for _ in range(DB)
                ]
                for dh in range(DB):
                    nc.vector.tensor_copy(sT_sb[dh][:], tps2[dh][:])

                if not is_last:
                    # ---- LN(source + x0) -> last (bf16), AllGather ----
                    lastl = spool.tile(
                        [P, MB, D], BF16, name="lastl", tag="t_loc"
                    )
                    for mb in range(MB):
                        xr = spool.tile(
                            [P, D], F32, name="xr", tag="xr", bufs=2
                        )
                        for dh in range(DB):
                            tr = ppool.tile(
                                [P, P], BF16, name="trs", tag="ps_tr", bufs=4
                            )
                            nc.tensor.transpose(
                                tr[:],
                                sT_sb[dh][:, mb * P : (mb + 1) * P],
                                ident[:],
                            )
                            nc.vector.tensor_add(
                                xr[:, dh * P : (dh + 1) * P],
                                tr[:],
                                x0_sb[:, mb, dh * P : (dh + 1) * P],
                            )
                        st = spool.tile(
                            [P, 4], F32, name="st", tag="st", bufs=2
                        )
                        nc.vector.reduce_sum(st[:, 0:1], xr[:], axis=AX)
                        nc.scalar.activation(
                            st[:, 1:2], st[:, 0:1], ACT.Copy, scale=1.0 / D
                        )
                        nc.vector.tensor_scalar_sub(xr[:], xr[:], st[:, 1:2])
                        sq = spool.tile(
                            [P, D], F32, name="sq", tag="mean_s", bufs=1
                        )
                        nc.scalar.square(sq[:], xr[:])
                        nc.vector.reduce_sum(st[:, 2:3], sq[:], axis=AX)
                        # std = sqrt(var + eps)
                        nc.scalar.activation(
                            st[:, 3:4],
                            st[:, 2:3],
                            ACT.Sqrt,
                            bias=eps_sb[:],
                            scale=1.0 / D,
                        )
                        nc.vector.reciprocal(st[:, 0:1], st[:, 3:4])
                        # y = ((x-mu)*rstd) * gamma + beta   (bf16 out)
                        nc.vector.scalar_tensor_tensor(
                            xr[:], xr[:], st[:, 0:1], gamma_sb, mult, mult
                        )
                        nc.vector.tensor_tensor(
                            lastl[:, mb, :], xr[:], beta_sb, add
                        )
                    nc.scalar.dma_start(
                        last_ag_in.rearrange("(mb p) d -> p mb d", p=P),
                        lastl[:],
                    )
                    nc.gpsimd.collective_compute(
                        "AllGather",
                        bypass,
                        replica_groups=rg,
                        ins=[last_ag_in[:].opt()],
                        outs=[last_ag_out[l][:].opt()],
                    )
                else:
                    # ---- gather pre-norm source for the hyperedge stage ----
                    srcl = spool.tile(
                        [P, MB, D], BF16, name="srcl", tag="t_loc"
                    )
                    for mb in range(MB):
                        for dh in range(DB):
                            tr = ppool.tile(
                                [P, P], BF16, name="trs", tag="ps_tr", bufs=4
                            )
                            nc.tensor.transpose(
                                tr[:],
                                sT_sb[dh][:, mb * P : (mb + 1) * P],
                                ident[:],
                            )
                            nc.vector.tensor_copy(
                                srcl[:, mb, dh * P : (dh + 1) * P], tr[:]
                            )
                    nc.scalar.dma_start(
                        src_ag_in.rearrange("(mb p) d -> p mb d", p=P),
                        srcl[:],
                    )
                    nc.gpsimd.collective_compute(
                        "AllGather",
                        bypass,
                        replica_groups=rg,
                        ins=[src_ag_in[:].opt()],
                        outs=[src_full[:].opt()],
                    )

            # ---------------- hyperedge masked mean + max ----------------
            # sums.T[d, e] = sum_n source[n, d] * H[n, e]; counts[e] = sum_n H[n, e]
            psA = [
                ppool.tile([P, El], F32, name=f"psA_{db}", tag=f"ps_t{db}")
                for db in range(DB)
            ]
            psC = ppool.tile([1, El], F32, name="psC", tag="ps_tr", bufs=4)
            src_r = src_full.rearrange("(c j p) d -> p c j d", p=P, j=CH)
            h_r = h_bf.rearrange("(c j p) e -> p c j e", p=P, j=CH)
            CJ = CH // 2
            for c in range(NCH):
                srcch = spool.tile(
                    [P, CH, D], BF16, name="srcch", tag="rhs", bufs=3
                )
                nc.scalar.dma_start(srcch[:], src_r[:, c, :, :])
                for half in range(2):
                    hch = spool.tile(
                        [P, CJ, El], BF16, name="hch", tag="hch", bufs=2
                    )
                    nc.sync.dma_start(
                        hch[:], h_r[:, c, half * CJ : (half + 1) * CJ, :]
                    )
                    for j2 in range(CJ):
                        j = half * CJ + j2
                        kb = c * CH + j
                        for db in range(DB):
                            nc.tensor.matmul(
                                psA[db][:],
                                srcch[:, j, db * P : (db + 1) * P],
                                hch[:, j2, :],
                                start=(kb == 0),
                                stop=(kb == KB - 1),
                            )
                        nc.tensor.matmul(
                            psC[:],
                            ones_sb[:],
                            hch[:, j2, :],
                            start=(kb == 0),
                            stop=(kb == KB - 1),
                        )

            # means.T = sums.T * (1/counts); local max over the edge shard
            crow = cpool.tile([1, El], F32, name="crow")
            nc.vector.reciprocal(crow[:], psC[:])
            cbc = cpool.tile([P, El], F32, name="cbc")
            nc.gpsimd.partition_broadcast(cbc[:], crow[:])
            mx = cpool.tile([P, 2 * DB], F32, name="mx")
            for db in range(DB):
                mean_s = spool.tile(
                    [P, El], F32, name="mean_s", tag="mean_s", bufs=1
                )
                nc.vector.tensor_tensor(mean_s[:], psA[db][:], cbc[:], mult)
                nc.vector.reduce_max(mx[:, db : db + 1], mean_s[:], axis=AX)
                nc.scalar.dma_start(
                    armax_in[:, db * P : (db + 1) * P].rearrange(
                        "one p -> p one"
                    ),
                    mx[:, db : db + 1],
                )

            # cross-core max of the per-shard maxima
            nc.gpsimd.collective_compute(
                "AllReduce",
                amax,
                replica_groups=rg,
                ins=[armax_in[:].opt()],
                outs=[armax_out[:].opt()],
            )
            nc.scalar.dma_start(out[None, :], armax_out[:])

    nc.compile()
    return nc


_CACHE = {}


def _get_program(N, D, E, n_layers, n_cores):
    key = (N, D, E, n_layers, n_cores)
    if key not in _CACHE:
        _CACHE[key] = build_program(N, D, E, n_layers, n_cores)
    return _CACHE[key]


def make_in_maps(node_embeddings, target_martrix, hypergraph_matrix,
                 ln_gamma, ln_beta, n_cores):
    N, D = node_embeddings.shape
    E = hypergraph_matrix.shape[1]
    Nl, El = N // n_cores, E // n_cores
    x0 = np.ascontiguousarray(node_embeddings, dtype=np.float32)
    A = np.asarray(target_martrix, dtype=np.float32)
    H = np.asarray(hypergraph_matrix, dtype=np.float32)
    in_maps = []
    for i in range(n_cores):
        rows = slice(i * Nl, (i + 1) * Nl)
        es = slice(i * El, (i + 1) * El)
        in_maps.append(
            {
                "a_col": np.ascontiguousarray(A[:, rows]),
                "a_row": np.ascontiguousarray(A[rows, :]),
                "x0_full": x0,
                "x0_loc": np.ascontiguousarray(x0[rows]),
                "hcol": np.ascontiguousarray(H[:, es]),
                "gamma": np.ascontiguousarray(ln_gamma, dtype=np.float32),
                "beta": np.ascontiguousarray(ln_beta, dtype=np.float32),
            }
        )
    return in_maps


def run(inputs, trace=False, n_cores=8, **run_kwargs):
    """Run on hardware; returns (full_output, BassKernelResults)."""
    node_embeddings = np.asarray(inputs["node_embeddings"], dtype=np.float32)
    target_martrix = np.asarray(inputs["target_martrix"], dtype=np.float32)
    hypergraph_matrix = np.asarray(
        inputs["hypergraph_matrix"], dtype=np.float32
    )
    ln_gamma = np.asarray(inputs["ln_gamma"], dtype=np.float32)
    ln_beta = np.asarray(inputs["ln_beta"], dtype=np.float32)
    n_layers = int(inputs["num_layers"])

    N, D = node_embeddings.shape
    E = hypergraph_matrix.shape[1]
    nc = _get_program(N, D, E, n_layers, n_cores)
    in_maps = make_in_maps(
        node_embeddings, target_martrix, hypergraph_matrix,
        ln_gamma, ln_beta, n_cores,
    )
    res = bass_utils.run_bass_kernel_spmd(
        nc, in_maps, core_ids=list(range(n_cores)), trace=trace, **run_kwargs
    )
    outs = np.stack([r["out"] for r in res.results])  # [n_cores, D]
    # every core holds the AllReduce(max) result; the max over cores is
    # identical and doubles as the gather step
    return np.max(outs, axis=0).astype(np.float32), res


def kernel(**inputs) -> np.ndarray:
    out, _ = run(inputs, trace=False)
    return out


# revision 44
# speedup vs baseline: 1.2007x; 1.2007x over previous
"""Trainium2 Bass kernel for nn_CasualGraph (segment_reduce).

Computes, on 8 NeuronCores:
    last = x0
    for l in range(num_layers):
        t      = A @ last
        source = A.T @ t
        last   = LN(source + x0)
    Hb    = (H > 0)
    means = (Hb.T @ source) / Hb.sum(0)[:, None]
    out   = means.max(axis=0)            # [D]

Sharding (8 cores, data-parallel over node rows):
  core i owns rows_i = rows [i*N/8, (i+1)*N/8).
  - pass1: t[rows_i] = A[rows_i, :] @ last    (lhsT slabs produced by DMA-transpose
           of a bf16, column-chunked copy of the row shard staged in DRAM)
  - AllGather(t)  [bf16, N/8 x D per rank]
  - pass2: source[rows_i] = A[:, rows_i].T @ t  (bf16 column shard resident in SBUF)
  - LN + residual locally, AllGather(last)
  Final layer gathers `source` instead, then each core computes the masked-mean
  over its own E/8 hyperedge shard ([source | 1].T @ H on PE) and the cross-core
  max is an AllReduce(max).

All matmuls run in bf16 (1 cycle/row on PE vs 4 for fp32) with fp32 PSUM
accumulation; numpy-simulated max relative error vs the fp32 reference is ~2.6e-3.
"""

import os
import sys

for _p in ("/opt/trn_rl_repo", os.path.expanduser("~/.axon_site/_ro/trn_rl_repo")):
    if os.path.isdir(_p) and _p not in sys.path:
        sys.path.insert(0, _p)

import ml_dtypes
import numpy as np

from concourse import bacc, bass, bass_utils, mybir, tile

F32 = mybir.dt.float32
BF16 = mybir.dt.bfloat16
P = 128  # SBUF/PSUM partitions


def build_program(N, D, E, n_layers, n_cores):
    """Build the SPMD Bass/Tile program (same program on every core)."""
    Nl = N // n_cores      # local rows per core
    MB = Nl // P           # output row-blocks per core
    KB = N // P            # contraction blocks over full N
    CH = 8                 # k-blocks per rhs streaming chunk
    NCH = KB // CH
    El = E // n_cores      # hyperedge shard
    DB = D // P            # D row-blocks (for the transposed segment-sum)
    assert Nl % P == 0 and N % P == 0 and KB % CH == 0 and D % P == 0
    eps = 1e-5

    QW = 256               # a_row columns per setup chunk
    NQ = N // QW
    KPQ = QW // P          # k-blocks per setup chunk
    NH = MB // 2           # PSUM accumulation tiles (2 row-blocks per bank)
    assert MB % 2 == 0

    nc = bacc.Bacc(
        "TRN2", target_bir_lowering=False, debug=False, num_devices=n_cores
    )
    ident_np = np.eye(P, dtype=np.float32)
    ident_dram = nc.inline_tensor(
        ident_np.astype(ml_dtypes.bfloat16), name="ident"
    )
    ident_f32_dram = nc.inline_tensor(ident_np, name="ident_f32")

    a_col = nc.dram_tensor("a_col", [N, Nl], F32, kind="ExternalInput").ap()
    a_row = nc.dram_tensor("a_row", [Nl, N], F32, kind="ExternalInput").ap()
    x0_full = nc.dram_tensor("x0_full", [N, D], F32, kind="ExternalInput").ap()
    x0_loc = nc.dram_tensor("x0_loc", [Nl, D], F32, kind="ExternalInput").ap()
    hcol = nc.dram_tensor("hcol", [N, El], F32, kind="ExternalInput").ap()
    gamma = nc.dram_tensor("gamma", [D], F32, kind="ExternalInput").ap()
    beta = nc.dram_tensor("beta", [D], F32, kind="ExternalInput").ap()
    out = nc.dram_tensor("out", [D], F32, kind="ExternalOutput").ap()

    rg = [list(range(n_cores))]
    bypass = mybir.AluOpType.bypass
    add = mybir.AluOpType.add
    mult = mybir.AluOpType.mult
    amax = mybir.AluOpType.max
    AX = mybir.AxisListType.X
    ACT = mybir.ActivationFunctionType

    with tile.TileContext(nc) as tc:
        with (
            tc.tile_pool(name="dram", bufs=1, space="DRAM") as dpool,
            tc.tile_pool(name="const", bufs=1) as cpool,
            tc.tile_pool(name="acolp", bufs=1) as acol_pool,
            tc.tile_pool(name="stream", bufs=1) as spool,
            tc.tile_pool(name="psum", bufs=1, space="PSUM") as ppool,
        ):
            # ---------------- DRAM staging buffers ----------------
            # Pre-transposed bf16 row shard: at_dram[kb] = A[rows_i, kb*P:(kb+1)*P].T
            # stored as contiguous [P, Nl] slabs (pass1 lhsT, plain streaming reads).
            at_dram = dpool.tile([KB // 2, P, 2 * Nl], BF16, name="at_dram")
            x0_bf = dpool.tile([N, D], BF16, name="x0_bf")
            # Shared collective outputs must each have a single writer:
            # one AllGather output buffer per collective.
            t_ag_in = dpool.tile([Nl, D], BF16, name="t_ag_in")
            t_ag_out = [
                dpool.tile(
                    [N, D], BF16, name=f"t_ag_out_{l}", addr_space="Shared"
                )
                for l in range(n_layers)
            ]
            last_ag_in = dpool.tile([Nl, D], BF16, name="last_ag_in")
            last_ag_out = [
                dpool.tile(
                    [N, D], BF16, name=f"last_ag_out_{l}", addr_space="Shared"
                )
                for l in range(n_layers - 1)
            ]
            src_ag_in = dpool.tile([Nl, D], BF16, name="src_ag_in")
            src_full = dpool.tile(
                [N, D], BF16, name="src_full", addr_space="Shared"
            )
            h_bf = dpool.tile([N, El], BF16, name="h_bf")
            armax_in = dpool.tile([1, D], F32, name="armax_in")
            armax_out = dpool.tile(
                [1, D], F32, name="armax_out", addr_space="Shared"
            )

            # ---------------- constants ----------------
            ident = cpool.tile([P, P], BF16, name="ident")
            nc.sync.dma_start(ident[:], ident_dram.ap())
            ident_f32 = cpool.tile([P, P], F32, name="ident_f32")
            nc.sync.dma_start(ident_f32[:], ident_f32_dram.ap())
            x0_sb = cpool.tile([P, MB, D], F32, name="x0_sb")
            nc.scalar.dma_start(
                x0_sb[:], x0_loc.rearrange("(mb p) d -> p mb d", p=P)
            )
            gb_row = cpool.tile([1, 2 * D], F32, name="gb_row")
            nc.scalar.dma_start(gb_row[:, 0:D], gamma[None, :])
            nc.scalar.dma_start(gb_row[:, D : 2 * D], beta[None, :])
            gb_sb = cpool.tile([P, 2 * D], F32, name="gb_sb")
            nc.gpsimd.partition_broadcast(gb_sb[:], gb_row[:])
            gamma_sb = gb_sb[:, 0:D]
            beta_sb = gb_sb[:, D : 2 * D]
            ones_sb = cpool.tile([P, 1], BF16, name="ones_sb")
            nc.vector.memset(ones_sb[:], 1.0)
            eps_sb = cpool.tile([P, 1], F32, name="eps_sb")
            nc.vector.memset(eps_sb[:], eps)

            # ---------------- setup casts (SWDGE FIFO order matters) ----
            # x0 (bf16) is layer-0 "last": gates the first rhs chunks
            nc.gpsimd.dma_start(x0_bf[:], x0_full[:])

            def load_rhs(src_r, c, tag):
                rhs = spool.tile(
                    [P, CH, D], BF16, name="rhs", tag="rhs", bufs=2
                )
                nc.scalar.dma_start(rhs[:], src_r[:, c, :, :])
                return rhs

            acol_sb = acol_pool.tile([P, KB, Nl], BF16, name="acol_sb")
            a_col_r = a_col.rearrange("(kb p) m -> p kb m", p=P)

            # ---------------- propagation layers ----------------
            # Both passes run "flipped": the stationary operand is the small
            # [128, D] rhs-chunk half (2 LDWEIGHTS per k-block) and the moving
            # operand is the wide A slab (N=512), producing transposed outputs
            # t.T / source.T in two [P, Nl] psum tiles (one accumulation group
            # per bank). Outputs are PE-transposed back to natural layout.
            # Layer 0's pass1 is fused with setup: a_row is cast-loaded in
            # column chunks and PE-transposed into lhsT slabs that feed both
            # at_dram (for layers 1+) and layer-0's matmuls directly.
            MH = max(Nl // 512, 1)
            MW = Nl // MH

            def pass_mms(tps, slab, rhs, j, kb):
                for dh in range(DB):
                    for mh in range(MH):
                        nc.tensor.matmul(
                            tps[dh][:, mh * MW : (mh + 1) * MW],
                            rhs[:, j, dh * P : (dh + 1) * P],
                            slab[:, mh * MW : (mh + 1) * MW],
                            start=(kb == 0),
                            stop=(kb == KB - 1),
                        )

            for l in range(n_layers):
                is_last = l == n_layers - 1

                # ---- pass1: t[rows_i] = A[rows_i, :] @ last  (as t.T) ----
                tps1 = [
                    ppool.tile([P, Nl], F32, name=f"tps1_{dh}", tag=f"ps_t{dh}")
                    for dh in range(DB)
                ]
                if l == 1:
                    # H shard -> bf16 staging; descriptors enqueue after the
                    # layer-0 collectives so the transfer rides under layer 1
                    nc.gpsimd.dma_start(h_bf[:], hcol[:])
                last_in = x0_bf if l == 0 else last_ag_out[l - 1]
                last_r = last_in.rearrange("(c j p) d -> p c j d", p=P, j=CH)
                rhs1 = None
                if l == 0:
                    # Setup fused with layer-0 pass1. a_row and a_col are
                    # loaded as plain fp32 on the two HWDGE rings (SWDGE casts
                    # are too slow); the cast happens on PE (transpose of the
                    # fp32 chunk straight into a bf16 psum tile) and DVE.
                    for kb in range(KB):
                        if kb % CH == 0:
                            rhs1 = load_rhs(last_r, kb // CH, "rhs1")
                        # fp32 column chunk of a_row for this k-block
                        arch = spool.tile(
                            [P, MB, P], F32, name="arch", tag="arch", bufs=2
                        )
                        nc.sync.dma_start(
                            arch[:],
                            a_row.rearrange("(mb p) (kb c) -> p mb kb c",
                                            p=P, c=P)[:, :, kb, :],
                        )
                        # fp32 row block of a_col -> bf16 resident via DVE
                        acch = spool.tile(
                            [P, Nl], F32, name="acch", tag="acch", bufs=2
                        )
                        nc.scalar.dma_start(acch[:], a_col_r[:, kb, :])
                        nc.vector.tensor_copy(acol_sb[:, kb, :], acch[:])
                        slab = spool.tile(
                            [P, Nl], BF16, name="slab", tag="slab", bufs=3
                        )
                        for mb in range(MB):
                            tr = ppool.tile(
                                [P, P], F32, name="tr", tag="ps_tr",
                                bufs=4,
                            )
                            nc.tensor.transpose(
                                tr[:], arch[:, mb, :], ident_f32[:]
                            )
                            nc.vector.tensor_copy(
                                slab[:, mb * P : (mb + 1) * P], tr[:]
                            )
                        nc.scalar.dma_start(
                            at_dram[kb // 2][
                                :, (kb % 2) * Nl : (kb % 2 + 1) * Nl
                            ],
                            slab[:],
                        )
                        pass_mms(tps1, slab, rhs1, kb % CH, kb)
                else:
                    for kb2 in range(KB // 2):
                        kb0 = kb2 * 2
                        if kb0 % CH == 0:
                            rhs1 = load_rhs(last_r, kb0 // CH, "rhs1")
                        slab2 = spool.tile(
                            [P, 2 * Nl], BF16, name="slab2", tag="slab2",
                            bufs=3,
                        )
                        nc.sync.dma_start(slab2[:], at_dram[kb2])
                        for q2 in range(2):
                            kb = kb0 + q2
                            pass_mms(
                                tps1,
                                slab2[:, q2 * Nl : (q2 + 1) * Nl],
                                rhs1,
                                kb % CH,
                                kb,
                            )

                # transpose t.T back to natural bf16 tiles and AllGather
                t_loc = spool.tile([P, MB, D], BF16, name="t_loc", tag="t_loc")
                tT_sb = [
                    spool.tile([P, Nl], BF16, name="tTs", tag="tTs", bufs=2)
                    for _ in range(DB)
                ]
                for dh in range(DB):
                    nc.vector.tensor_copy(tT_sb[dh][:], tps1[dh][:])
                for mb in range(MB):
                    for dh in range(DB):
                        tr = ppool.tile(
                            [P, P], BF16, name="trb", tag="ps_tr", bufs=4
                        )
                        nc.tensor.transpose(
                            tr[:],
                            tT_sb[dh][:, mb * P : (mb + 1) * P],
                            ident[:],
                        )
                        nc.vector.tensor_copy(
                            t_loc[:, mb, dh * P : (dh + 1) * P], tr[:]
                        )
                nc.scalar.dma_start(
                    t_ag_in.rearrange("(mb p) d -> p mb d", p=P), t_loc[:]
                )
                nc.gpsimd.collective_compute(
                    "AllGather",
                    bypass,
                    replica_groups=rg,
                    ins=[t_ag_in[:].opt()],
                    outs=[t_ag_out[l][:].opt()],
                )

                # ---- pass2: source[rows_i] = A[:, rows_i].T @ t (as src.T) ----
                tps2 = [
                    ppool.tile([P, Nl], F32, name=f"tps2_{dh}", tag=f"ps_t{dh}")
                    for dh in range(DB)
                ]
                t_r = t_ag_out[l].rearrange("(c j p) d -> p c j d", p=P, j=CH)
                rhs2 = None
                for kb in range(KB):
                    if kb % CH == 0:
                        rhs2 = load_rhs(t_r, kb // CH, "rhs2")
                    pass_mms(tps2, acol_sb[:, kb, :], rhs2, kb % CH, kb)

                # transpose src.T back (bf16), fusing the +x0 residual
                sT_sb = [
                    spool.tile([P, Nl], BF16, name="sTs", tag="sTs", bufs=2)
                    for _ in range(DB)
                ]
                for dh in range(DB):
                    nc.vector.tensor_copy(sT_sb[dh][:], tps2[dh][:])

                if not is_last:
                    # ---- LN(source + x0) -> last (bf16), AllGather ----
                    lastl = spool.tile(
                        [P, MB, D], BF16, name="lastl", tag="t_loc"
                    )
                    for mb in range(MB):
                        xr = spool.tile(
                            [P, D], F32, name="xr", tag="xr", bufs=2
                        )
                        for dh in range(DB):
                            tr = ppool.tile(
                                [P, P], BF16, name="trs", tag="ps_tr", bufs=4
                            )
                            nc.tensor.transpose(
                                tr[:],
                                sT_sb[dh][:, mb * P : (mb + 1) * P],
                                ident[:],
                            )
                            nc.vector.tensor_add(
                                xr[:, dh * P : (dh + 1) * P],
                                tr[:],
                                x0_sb[:, mb, dh * P : (dh + 1) * P],
                            )
                        st = spool.tile(
                            [P, 4], F32, name="st", tag="st", bufs=2
                        )
                        nc.vector.reduce_sum(st[:, 0:1], xr[:], axis=AX)
                        nc.scalar.activation(
                            st[:, 1:2], st[:, 0:1], ACT.Copy, scale=1.0 / D
                        )
                        nc.vector.tensor_scalar_sub(xr[:], xr[:], st[:, 1:2])
                        sq = spool.tile(
                            [P, D], F32, name="sq", tag="mean_s", bufs=1
                        )
                        nc.scalar.square(sq[:], xr[:])
                        nc.vector.reduce_sum(st[:, 2:3], sq[:], axis=AX)
                        # std = sqrt(var + eps)
                        nc.scalar.activation(
                            st[:, 3:4],
                            st[:, 2:3],
                            ACT.Sqrt,
                            bias=eps_sb[:],
                            scale=1.0 / D,
                        )
                        nc.vector.reciprocal(st[:, 0:1], st[:, 3:4])
                        # y = ((x-mu)*rstd) * gamma + beta   (bf16 out)
                        nc.vector.scalar_tensor_tensor(
                            xr[:], xr[:], st[:, 0:1], gamma_sb, mult, mult
                        )
                        nc.vector.tensor_tensor(
                            lastl[:, mb, :], xr[:], beta_sb, add
                        )
                    nc.scalar.dma_start(
                        last_ag_in.rearrange("(mb p) d -> p mb d", p=P),
                        lastl[:],
                    )
                    nc.gpsimd.collective_compute(
                        "AllGather",
                        bypass,
                        replica_groups=rg,
                        ins=[last_ag_in[:].opt()],
                        outs=[last_ag_out[l][:].opt()],
                    )
                else:
                    # ---- gather pre-norm source for the hyperedge stage ----
                    srcl = spool.tile(
                        [P, MB, D], BF16, name="srcl", tag="t_loc"
                    )
                    for mb in range(MB):
                        for dh in range(DB):
                            tr = ppool.tile(
                                [P, P], BF16, name="trs", tag="ps_tr", bufs=4
                            )
                            nc.tensor.transpose(
                                tr[:],
                                sT_sb[dh][:, mb * P : (mb + 1) * P],
                                ident[:],
                            )
                            nc.vector.tensor_copy(
                                srcl[:, mb, dh * P : (dh + 1) * P], tr[:]
                            )
                    nc.scalar.dma_start(
                        src_ag_in.rearrange("(mb p) d -> p mb d", p=P),
                        srcl[:],
                    )
                    nc.gpsimd.collective_compute(
                        "AllGather",
                        bypass,
                        replica_groups=rg,
                        ins=[src_ag_in[:].opt()],
                        outs=[src_full[:].opt()],
                    )

            # ---------------- hyperedge masked mean + max ----------------
            # sums.T[d, e] = sum_n source[n, d] * H[n, e]; counts[e] = sum_n H[n, e]
            psA = [
                ppool.tile([P, El], F32, name=f"psA_{db}", tag=f"ps_t{db}")
                for db in range(DB)
            ]
            psC = ppool.tile([1, El], F32, name="psC", tag="ps_tr", bufs=4)
            src_r = src_full.rearrange("(c j p) d -> p c j d", p=P, j=CH)
            h_r = h_bf.rearrange("(c j p) e -> p c j e", p=P, j=CH)
            CJ = 2
            for c in range(NCH):
                srcch = spool.tile(
                    [P, CH, D], BF16, name="srcch", tag="rhs", bufs=2
                )
                nc.scalar.dma_start(srcch[:], src_r[:, c, :, :])
                for half in range(CH // CJ):
                    hch = spool.tile(
                        [P, CJ, El], BF16, name="hch", tag="hch", bufs=2
                    )
                    nc.sync.dma_start(
                        hch[:], h_r[:, c, half * CJ : (half + 1) * CJ, :]
                    )
                    for j2 in range(CJ):
                        j = half * CJ + j2
                        kb = c * CH + j
                        for db in range(DB):
                            nc.tensor.matmul(
                                psA[db][:],
                                srcch[:, j, db * P : (db + 1) * P],
                                hch[:, j2, :],
                                start=(kb == 0),
                                stop=(kb == KB - 1),
                            )
                        nc.tensor.matmul(
                            psC[:],
                            ones_sb[:],
                            hch[:, j2, :],
                            start=(kb == 0),
                            stop=(kb == KB - 1),
                        )

            # means.T = sums.T * (1/counts); local max over the edge shard
            crow = cpool.tile([1, El], F32, name="crow")
            nc.vector.reciprocal(crow[:], psC[:])
            cbc = cpool.tile([P, El], F32, name="cbc")
            nc.gpsimd.partition_broadcast(cbc[:], crow[:])
            mx = cpool.tile([P, 2 * DB], F32, name="mx")
            for db in range(DB):
                mean_s = spool.tile(
                    [P, El], F32, name="mean_s", tag="mean_s", bufs=1
                )
                nc.vector.tensor_tensor(mean_s[:], psA[db][:], cbc[:], mult)
                nc.vector.reduce_max(mx[:, db : db + 1], mean_s[:], axis=AX)
                nc.scalar.dma_start(
                    armax_in[:, db * P : (db + 1) * P].rearrange(
                        "one p -> p one"
                    ),
                    mx[:, db : db + 1],
                )

            # cross-core max of the per-shard maxima
            nc.gpsimd.collective_compute(
                "AllReduce",
                amax,
                replica_groups=rg,
                ins=[armax_in[:].opt()],
                outs=[armax_out[:].opt()],
            )
            nc.scalar.dma_start(out[None, :], armax_out[:])

    nc.compile()
    return nc


_CACHE = {}


def _get_program(N, D, E, n_layers, n_cores):
    key = (N, D, E, n_layers, n_cores)
    if key not in _CACHE:
        _CACHE[key] = build_program(N, D, E, n_layers, n_cores)
    return _CACHE[key]


def make_in_maps(node_embeddings, target_martrix, hypergraph_matrix,
                 ln_gamma, ln_beta, n_cores):
    N, D = node_embeddings.shape
    E = hypergraph_matrix.shape[1]
    Nl, El = N // n_cores, E // n_cores
    x0 = np.ascontiguousarray(node_embeddings, dtype=np.float32)
    A = np.asarray(target_martrix, dtype=np.float32)
    H = np.asarray(hypergraph_matrix, dtype=np.float32)
    in_maps = []
    for i in range(n_cores):
        rows = slice(i * Nl, (i + 1) * Nl)
        es = slice(i * El, (i + 1) * El)
        in_maps.append(
            {
                "a_col": np.ascontiguousarray(A[:, rows]),
                "a_row": np.ascontiguousarray(A[rows, :]),
                "x0_full": x0,
                "x0_loc": np.ascontiguousarray(x0[rows]),
                "hcol": np.ascontiguousarray(H[:, es]),
                "gamma": np.ascontiguousarray(ln_gamma, dtype=np.float32),
                "beta": np.ascontiguousarray(ln_beta, dtype=np.float32),
            }
        )
    return in_maps


def run(inputs, trace=False, n_cores=8, **run_kwargs):
    """Run on hardware; returns (full_output, BassKernelResults)."""
    node_embeddings = np.asarray(inputs["node_embeddings"], dtype=np.float32)
    target_martrix = np.asarray(inputs["target_martrix"], dtype=np.float32)
    hypergraph_matrix = np.asarray(
        inputs["hypergraph_matrix"], dtype=np.float32
    )
    ln_gamma = np.asarray(inputs["ln_gamma"], dtype=np.float32)
    ln_beta = np.asarray(inputs["ln_beta"], dtype=np.float32)
    n_layers = int(inputs["num_layers"])

    N, D = node_embeddings.shape
    E = hypergraph_matrix.shape[1]
    nc = _get_program(N, D, E, n_layers, n_cores)
    in_maps = make_in_maps(
        node_embeddings, target_martrix, hypergraph_matrix,
        ln_gamma, ln_beta, n_cores,
    )
    res = bass_utils.run_bass_kernel_spmd(
        nc, in_maps, core_ids=list(range(n_cores)), trace=trace, **run_kwargs
    )
    outs = np.stack([r["out"] for r in res.results])  # [n_cores, D]
    # every core holds the AllReduce(max) result; the max over cores is
    # identical and doubles as the gather step
    return np.max(outs, axis=0).astype(np.float32), res


def kernel(**inputs) -> np.ndarray:
    out, _ = run(inputs, trace=False)
    return out
